# revision 8
# baseline (speedup 1.0000x reference)
"""Trainium2 Bass kernel for nn_DenseModel_51926154609008 (weighted-rank
contrastive CE loss).

Math (reference semantics, no sort needed):
  scores = q @ p.T                       [B=2048, P=16384]
  t_i    = scores[i, 8*i]                (positive/target score)
  rank_i = #{j : scores[i, j] > t_i}     (argsort position == exceed count,
                                          ties are measure-zero for randn data)
  lse_i  = logsumexp(scores[i, :])
  loss   = mean((lse_i - t_i) * (1 + 2.6*exp(-(rank_i-1)^2 / (2*1.8^2))))

Sharding: passage-parallel (P split across 8 cores, q replicated). Each core
computes a [2048, 2048] score slab and per-query partial statistics:
  sumexp_c[i] = sum_j exp(s_ij - C)      (fixed shift C, so partials add
                                          across cores without a max-merge)
  cnt_c[i]    = #{j in slab : s_ij > t_i}
The host then combines partials and evaluates the tiny [2048] tail in fp64.

The self-comparison (j == 8i) must contribute exactly 0 to rank_i. Query i's
target column lives only in core (i//256)'s slab. Each core rotates the query
order (data-level permutation, program stays SPMD-uniform) so its own queries
are always m-tiles 0 and 1; for those two tiles the count op compares against
a host-built threshold tensor with +BIG at the self position (structural
exclusion); all other tiles compare against a per-partition t broadcast.

t itself is computed on the host (trivial 2048x768 row-dot).
"""

import sys

import numpy as np

sys.path.insert(0, "/opt/trn_rl_repo")

import concourse.bacc as bacc  # noqa: E402
import concourse.bass as bass  # noqa: E402
import concourse.mybir as mybir  # noqa: E402
import concourse.tile as tile  # noqa: E402
from concourse.bass_utils import run_bass_kernel_spmd  # noqa: E402

# Problem shape (hardcoded per the task contract).
B = 2048
D = 768
NP = 8
P = B * NP  # 16384
NCORES = 8
PSLAB = P // NCORES  # 2048 passage columns per core
KCH = D // 128  # 6 contraction chunks
MT = B // 128  # 16 query m-tiles
NT = PSLAB // 512  # 4 psum n-tiles
QSLAB = B // NCORES  # 256 queries owned per core

C_SHIFT = 128.0  # fixed exp shift: exp(s - C) never overflows for this data
BIG = np.float32(3.0e38)  # "+inf" for structural self-exclusion in counts

ALPHA = 2.6
OPTIMAL_RANK = 1.0
SIGMA = 1.8

# Matmul input dtype: "bfloat16" | "float32r" | "float32"
MM_DT = mybir.dt.float32r

_STATE: dict = {}


def _build_nc(mm_dt):
    nc = bacc.Bacc("TRN2", target_bir_lowering=False, debug=False,
                   num_devices=NCORES)

    qT_d = nc.dram_tensor("qT", [D, B], mm_dt, kind="ExternalInput").ap()
    pT_d = nc.dram_tensor("pT", [D, PSLAB], mm_dt, kind="ExternalInput").ap()
    tv_d = nc.dram_tensor("tvec", [128, MT], mybir.dt.float32,
                          kind="ExternalInput").ap()
    msk_d = nc.dram_tensor("msk", [128, 2 * PSLAB], mybir.dt.float32,
                           kind="ExternalInput").ap()
    se_d = nc.dram_tensor("se_out", [128, MT], mybir.dt.float32,
                          kind="ExternalOutput").ap()
    cnt_d = nc.dram_tensor("cnt_out", [128, MT * NT], mybir.dt.float32,
                           kind="ExternalOutput").ap()

    f32 = mybir.dt.float32
    bf16 = mybir.dt.bfloat16

    with tile.TileContext(nc) as tc:
        with (
            tc.tile_pool(name="weights", bufs=1) as wpool,
            tc.tile_pool(name="stats", bufs=1) as spool,
            tc.tile_pool(name="junk", bufs=2) as jpool,
            tc.tile_pool(name="psum", bufs=2,
                         space=bass.MemorySpace.PSUM) as ppool,
        ):
            qk = []
            pk = []
            for k in range(KCH):
                qt = wpool.tile([128, B], mm_dt, name=f"qk{k}", tag=f"qk{k}")
                pt = wpool.tile([128, PSLAB], mm_dt, name=f"pk{k}",
                                tag=f"pk{k}")
                nc.sync.dma_start(pt[:], pT_d[k * 128:(k + 1) * 128, :])
                nc.sync.dma_start(qt[:], qT_d[k * 128:(k + 1) * 128, :])
                qk.append(qt)
                pk.append(pt)
            tv = spool.tile([128, MT], f32, name="tv", tag="tv")
            msk = spool.tile([128, 2 * PSLAB], f32, name="msk", tag="msk")
            nc.sync.dma_start(tv[:], tv_d[:])
            nc.sync.dma_start(msk[:], msk_d[:])

            se_sb = spool.tile([128, MT], f32, name="se_sb", tag="se_sb")
            cnt_sb = spool.tile([128, MT * NT], f32, name="cnt_sb",
                                tag="cnt_sb")
            negc = spool.tile([128, 1], f32, name="negc", tag="negc")
            nc.vector.memset(negc[:], -C_SHIFT)

            for m in range(MT):
                ps = ppool.tile([128, PSLAB], f32, name="ps", tag="ps")
                for n in range(NT):
                    for k in range(KCH):
                        nc.tensor.matmul(
                            ps[:, n * 512:(n + 1) * 512],
                            qk[k][:, m * 128:(m + 1) * 128],
                            pk[k][:, n * 512:(n + 1) * 512],
                            start=(k == 0),
                            stop=(k == KCH - 1),
                        )
                je = jpool.tile([128, PSLAB], bf16, name="je", tag="je")
                nc.scalar.activation(
                    je[:], ps[:], mybir.ActivationFunctionType.Exp,
                    bias=negc[:], scale=1.0,
                    accum_out=se_sb[:, m:m + 1],
                )
                # DVE reads must not cross PSUM bank boundaries (HW fault)
                # -> one count op per 512-col bank.
                jc = jpool.tile([128, PSLAB], bf16, name="jc", tag="jc")
                for n in range(NT):
                    psn = ps[:, n * 512:(n + 1) * 512]
                    jcn = jc[:, n * 512:(n + 1) * 512]
                    cacc = cnt_sb[:, m * NT + n:m * NT + n + 1]
                    if m < 2:
                        nc.vector.scalar_tensor_tensor(
                            out=jcn, in0=psn, scalar=tv[:, m:m + 1],
                            in1=msk[:, m * PSLAB + n * 512:
                                    m * PSLAB + (n + 1) * 512],
                            op0=mybir.AluOpType.is_gt,
                            op1=mybir.AluOpType.mult,
                            accum_out=cacc,
                        )
                    else:
                        nc.vector.tensor_scalar(
                            jcn, psn, tv[:, m:m + 1], None,
                            op0=mybir.AluOpType.is_gt,
                            op1=mybir.AluOpType.add,
                            accum_out=cacc,
                        )

            nc.sync.dma_start(se_d[:], se_sb[:])
            nc.sync.dma_start(cnt_d[:], cnt_sb[:])

    nc.compile()
    return nc


def _np_dtype(mm_dt):
    if mm_dt == mybir.dt.bfloat16:
        import ml_dtypes
        return ml_dtypes.bfloat16
    return np.float32


def _perm(c):
    return np.concatenate([np.arange(c * QSLAB, B), np.arange(0, c * QSLAB)])


def prepare(q, p, mm_dt=None):
    """Host-side shard prep. Returns (in_maps, t32, perms)."""
    if mm_dt is None:
        mm_dt = MM_DT
    npdt = _np_dtype(mm_dt)
    q = np.ascontiguousarray(np.asarray(q, dtype=np.float32))
    p = np.ascontiguousarray(np.asarray(p, dtype=np.float32))

    # target scores t_i = q_i . p_{8i}, fp32 (matches reference's fp32 path
    # to ~1e-7; only used as compare threshold + in the tiny host tail)
    t32 = np.einsum("ij,ij->i", q, p[::NP], dtype=np.float64).astype(np.float32)

    qT = np.ascontiguousarray(q.T)  # [D, B] fp32
    r = np.arange(128)

    in_maps = []
    perms = []
    for c in range(NCORES):
        perm = _perm(c)
        perms.append(perm)
        qTc = np.ascontiguousarray(qT[:, perm]).astype(npdt)
        pTc = np.ascontiguousarray(p[c * PSLAB:(c + 1) * PSLAB].T).astype(npdt)
        tvc = np.ascontiguousarray(t32[perm].reshape(MT, 128).T)
        msk = np.ones((128, 2 * PSLAB), dtype=np.float32)
        msk[r, 8 * r] = 0.0                  # tile 0 self column
        msk[r, PSLAB + 1024 + 8 * r] = 0.0   # tile 1 self column
        in_maps.append({"qT": qTc, "pT": pTc, "tvec": tvc, "msk": msk})
    return in_maps, t32, perms


def finalize(results, t32, perms):
    """Combine per-core partials into the scalar loss (fp64 host tail)."""
    se_tot = np.zeros(B, dtype=np.float64)
    cnt_tot = np.zeros(B, dtype=np.float64)
    for c in range(NCORES):
        perm = perms[c]
        se_flat = results[c]["se_out"].astype(np.float64).T.ravel()
        cnt_flat = (results[c]["cnt_out"].astype(np.float64)
                    .reshape(128, MT, NT).sum(-1).T.ravel())
        se_tot[perm] += se_flat
        cnt_tot[perm] += cnt_flat
    lse = C_SHIFT + np.log(se_tot)
    raw = lse - t32.astype(np.float64)
    w = 1.0 + ALPHA * np.exp(-((cnt_tot - OPTIMAL_RANK) ** 2)
                             / (2.0 * SIGMA ** 2))
    return np.float32(np.mean(raw * w))


def _get_nc(mm_dt=None):
    if mm_dt is None:
        mm_dt = MM_DT
    if mm_dt not in _STATE:
        _STATE[mm_dt] = _build_nc(mm_dt)
    return _STATE[mm_dt]


def kernel(q_reps, p_reps, n_passages):
    assert int(np.asarray(n_passages)) == NP
    nc = _get_nc()
    in_maps, t32, perms = prepare(q_reps, p_reps)
    res = run_bass_kernel_spmd(nc, in_maps, core_ids=list(range(NCORES)))
    return finalize(res.results, t32, perms)


def run_profiled(q_reps, p_reps, n_passages, mm_dt=None, trace=True):
    """Same as kernel() but returns (loss, BassKernelResults) with NTFF
    profile (requires the antenv.axon_hooks shim; see _install_ntff_shim)."""
    nc = _get_nc(mm_dt)
    in_maps, t32, perms = prepare(q_reps, p_reps, mm_dt)
    # warm-up / correctness run (also warms the PJRT executable cache)
    res = run_bass_kernel_spmd(nc, in_maps, core_ids=list(range(NCORES)),
                               trace=trace)
    loss = finalize(res.results, t32, perms)
    return loss, res


def _install_ntff_shim():
    """Provide antenv.axon_hooks (absent in this image) so trace=True works."""
    import types
    import antenv
    if "antenv.axon_hooks" in sys.modules:
        return
    mod = types.ModuleType("antenv.axon_hooks")
    mod._hook = None
    mod.set_axon_ntff_profile_hook = lambda h: setattr(mod, "_hook", h)
    mod.get_axon_ntff_profile_hook = lambda: mod._hook
    sys.modules["antenv.axon_hooks"] = mod
    antenv.axon_hooks = mod
    try:
        from trn_agent_boot.trn_boot import _ntff_profile_via_ctypes
        hook = _ntff_profile_via_ctypes("/opt/axon/libaxon_pjrt.so")
        if hook is not None:
            mod._hook = hook
    except Exception:
        pass


# revision 13
# speedup vs baseline: 1.0732x; 1.0732x over previous
"""Trainium2 Bass kernel for nn_DenseModel_51926154609008 (weighted-rank
contrastive CE loss).

Math (reference semantics, no sort needed):
  scores = q @ p.T                       [B=2048, P=16384]
  t_i    = scores[i, 8*i]                (positive/target score)
  rank_i = #{j : scores[i, j] > t_i}     (argsort position == exceed count,
                                          ties are measure-zero for randn data)
  lse_i  = logsumexp(scores[i, :])
  loss   = mean((lse_i - t_i) * (1 + 2.6*exp(-(rank_i-1)^2 / (2*1.8^2))))

Sharding: passage-parallel (P split across 8 cores, q replicated). Each core
computes a [2048, 2048] score slab and per-query partial statistics:
  sumexp_c[i] = sum_j exp(s_ij - C)      (fixed shift C, so partials add
                                          across cores without a max-merge)
  cnt_c[i]    = #{j in slab : s_ij > t_i}
The host then combines partials and evaluates the tiny [2048] tail in fp64.

The self-comparison (j == 8i) must contribute exactly 0 to rank_i. Query i's
target column lives only in core (i//256)'s slab. Each core rotates the query
order (data-level permutation, program stays SPMD-uniform) so its own queries
are always m-tiles 0 and 1; for those two tiles the count op compares against
a host-built threshold tensor with +BIG at the self position (structural
exclusion); all other tiles compare against a per-partition t broadcast.

t itself is computed on the host (trivial 2048x768 row-dot).
"""

import sys

import numpy as np

sys.path.insert(0, "/opt/trn_rl_repo")

import concourse.bacc as bacc  # noqa: E402
import concourse.bass as bass  # noqa: E402
import concourse.mybir as mybir  # noqa: E402
import concourse.tile as tile  # noqa: E402
from concourse.bass_utils import run_bass_kernel_spmd  # noqa: E402

# Problem shape (hardcoded per the task contract).
B = 2048
D = 768
NP = 8
P = B * NP  # 16384
NCORES = 8
PSLAB = P // NCORES  # 2048 passage columns per core
KCH = D // 128  # 6 contraction chunks
MT = B // 128  # 16 query m-tiles
NT = PSLAB // 512  # 4 psum n-tiles
QSLAB = B // NCORES  # 256 queries owned per core

C_SHIFT = 128.0  # fixed exp shift: exp(s - C) never overflows for this data
BIG = np.float32(3.0e38)  # "+inf" for structural self-exclusion in counts

ALPHA = 2.6
OPTIMAL_RANK = 1.0
SIGMA = 1.8

# Matmul input dtype: "bfloat16" | "float32r" | "float32"
MM_DT = mybir.dt.bfloat16

# Count-engine split: tiles 0,1 are the core's own queries (masked,
# exact is_gt on DVE); tiles 2..SIG_SPLIT-1 exact is_gt on DVE;
# tiles SIG_SPLIT..15 run on ScalarE as saturated-sigmoid step counts
# (integer-exact except fractional contributions from |s-t| < ~1e-3
# mid-rank ties, which carry zero weight in the loss).
SIG_SPLIT = 11
SIG_SCALE = 30000.0

_STATE: dict = {}


def _build_nc(mm_dt):
    nc = bacc.Bacc("TRN2", target_bir_lowering=False, debug=False,
                   num_devices=NCORES)

    qT_d = nc.dram_tensor("qT", [D, B], mm_dt, kind="ExternalInput").ap()
    pT_d = nc.dram_tensor("pT", [D, PSLAB], mm_dt, kind="ExternalInput").ap()
    tv_d = nc.dram_tensor("tvec", [128, MT], mybir.dt.float32,
                          kind="ExternalInput").ap()
    sgb_d = nc.dram_tensor("sgb", [128, MT], mybir.dt.float32,
                           kind="ExternalInput").ap()
    msk_d = nc.dram_tensor("msk", [128, 2 * PSLAB], mybir.dt.float32,
                           kind="ExternalInput").ap()
    se_d = nc.dram_tensor("se_out", [128, MT], mybir.dt.float32,
                          kind="ExternalOutput").ap()
    cnt_d = nc.dram_tensor("cnt_out", [128, MT * NT], mybir.dt.float32,
                           kind="ExternalOutput").ap()

    f32 = mybir.dt.float32
    bf16 = mybir.dt.bfloat16

    with tile.TileContext(nc) as tc:
        with (
            tc.tile_pool(name="weights", bufs=1) as wpool,
            tc.tile_pool(name="stats", bufs=1) as spool,
            tc.tile_pool(name="junk", bufs=2) as jpool,
            tc.tile_pool(name="psum", bufs=2,
                         space=bass.MemorySpace.PSUM) as ppool,
        ):
            qk = []
            pk = []
            for k in range(KCH):
                qt = wpool.tile([128, B], mm_dt, name=f"qk{k}", tag=f"qk{k}")
                pt = wpool.tile([128, PSLAB], mm_dt, name=f"pk{k}",
                                tag=f"pk{k}")
                nc.sync.dma_start(pt[:], pT_d[k * 128:(k + 1) * 128, :])
                nc.sync.dma_start(qt[:], qT_d[k * 128:(k + 1) * 128, :])
                qk.append(qt)
                pk.append(pt)
            tv = spool.tile([128, MT], f32, name="tv", tag="tv")
            sgb = spool.tile([128, MT], f32, name="sgb", tag="sgb")
            msk = spool.tile([128, 2 * PSLAB], f32, name="msk", tag="msk")
            nc.sync.dma_start(tv[:], tv_d[:])
            nc.sync.dma_start(sgb[:], sgb_d[:])
            nc.sync.dma_start(msk[:], msk_d[:])

            se_sb = spool.tile([128, MT], f32, name="se_sb", tag="se_sb")
            cnt_sb = spool.tile([128, MT * NT], f32, name="cnt_sb",
                                tag="cnt_sb")
            nc.vector.memset(cnt_sb[:], 0.0)
            negc = spool.tile([128, 1], f32, name="negc", tag="negc")
            nc.vector.memset(negc[:], -C_SHIFT)

            for m in range(MT):
                ps = ppool.tile([128, PSLAB], f32, name="ps", tag="ps")
                for n in range(NT):
                    for k in range(KCH):
                        nc.tensor.matmul(
                            ps[:, n * 512:(n + 1) * 512],
                            qk[k][:, m * 128:(m + 1) * 128],
                            pk[k][:, n * 512:(n + 1) * 512],
                            start=(k == 0),
                            stop=(k == KCH - 1),
                        )
                je = jpool.tile([128, PSLAB], bf16, name="je", tag="je")
                nc.scalar.activation(
                    je[:], ps[:], mybir.ActivationFunctionType.Exp,
                    bias=negc[:], scale=1.0,
                    accum_out=se_sb[:, m:m + 1],
                )
                jc = jpool.tile([128, PSLAB], bf16, name="jc", tag="jc")
                if m >= SIG_SPLIT:
                    # step-function count on ScalarE (sigmoid saturates to
                    # exact 0/1 beyond |s-t| ~ 1e-3); 4-bank read is fine
                    # on ACT.
                    nc.scalar.activation(
                        jc[:], ps[:], mybir.ActivationFunctionType.Sigmoid,
                        bias=sgb[:, m:m + 1], scale=SIG_SCALE,
                        accum_out=cnt_sb[:, m * NT:m * NT + 1],
                    )
                else:
                    # DVE reads must not cross PSUM bank boundaries
                    # (HW fault) -> one count op per 512-col bank.
                    for n in range(NT):
                        psn = ps[:, n * 512:(n + 1) * 512]
                        jcn = jc[:, n * 512:(n + 1) * 512]
                        cacc = cnt_sb[:, m * NT + n:m * NT + n + 1]
                        if m < 2:
                            nc.vector.scalar_tensor_tensor(
                                out=jcn, in0=psn, scalar=tv[:, m:m + 1],
                                in1=msk[:, m * PSLAB + n * 512:
                                        m * PSLAB + (n + 1) * 512],
                                op0=mybir.AluOpType.is_gt,
                                op1=mybir.AluOpType.mult,
                                accum_out=cacc,
                            )
                        else:
                            nc.vector.tensor_scalar(
                                jcn, psn, tv[:, m:m + 1], None,
                                op0=mybir.AluOpType.is_gt,
                                op1=mybir.AluOpType.add,
                                accum_out=cacc,
                            )

            nc.sync.dma_start(se_d[:], se_sb[:])
            nc.sync.dma_start(cnt_d[:], cnt_sb[:])

    nc.compile()
    return nc


def _np_dtype(mm_dt):
    if mm_dt == mybir.dt.bfloat16:
        import ml_dtypes
        return ml_dtypes.bfloat16
    return np.float32


def _perm(c):
    return np.concatenate([np.arange(c * QSLAB, B), np.arange(0, c * QSLAB)])


def prepare(q, p, mm_dt=None):
    """Host-side shard prep. Returns (in_maps, t32, perms)."""
    if mm_dt is None:
        mm_dt = MM_DT
    npdt = _np_dtype(mm_dt)
    q = np.ascontiguousarray(np.asarray(q, dtype=np.float32))
    p = np.ascontiguousarray(np.asarray(p, dtype=np.float32))

    # target scores t_i = q_i . p_{8i}, fp32 (matches reference's fp32 path
    # to ~1e-7; only used as compare threshold + in the tiny host tail)
    t32 = np.einsum("ij,ij->i", q, p[::NP], dtype=np.float64).astype(np.float32)

    qT = np.ascontiguousarray(q.T)  # [D, B] fp32
    r = np.arange(128)

    in_maps = []
    perms = []
    for c in range(NCORES):
        perm = _perm(c)
        perms.append(perm)
        qTc = np.ascontiguousarray(qT[:, perm]).astype(npdt)
        pTc = np.ascontiguousarray(p[c * PSLAB:(c + 1) * PSLAB].T).astype(npdt)
        tvc = np.ascontiguousarray(t32[perm].reshape(MT, 128).T)
        msk = np.ones((128, 2 * PSLAB), dtype=np.float32)
        msk[r, 8 * r] = 0.0                  # tile 0 self column
        msk[r, PSLAB + 1024 + 8 * r] = 0.0   # tile 1 self column
        in_maps.append({"qT": qTc, "pT": pTc, "tvec": tvc,
                        "sgb": -SIG_SCALE * tvc, "msk": msk})
    return in_maps, t32, perms


def finalize(results, t32, perms):
    """Combine per-core partials into the scalar loss (fp64 host tail)."""
    se_tot = np.zeros(B, dtype=np.float64)
    cnt_tot = np.zeros(B, dtype=np.float64)
    for c in range(NCORES):
        perm = perms[c]
        se_flat = results[c]["se_out"].astype(np.float64).T.ravel()
        cnt_flat = (results[c]["cnt_out"].astype(np.float64)
                    .reshape(128, MT, NT).sum(-1).T.ravel())
        se_tot[perm] += se_flat
        cnt_tot[perm] += cnt_flat
    lse = C_SHIFT + np.log(se_tot)
    raw = lse - t32.astype(np.float64)
    w = 1.0 + ALPHA * np.exp(-((cnt_tot - OPTIMAL_RANK) ** 2)
                             / (2.0 * SIGMA ** 2))
    return np.float32(np.mean(raw * w))


def _get_nc(mm_dt=None):
    if mm_dt is None:
        mm_dt = MM_DT
    if mm_dt not in _STATE:
        _STATE[mm_dt] = _build_nc(mm_dt)
    return _STATE[mm_dt]


def kernel(q_reps, p_reps, n_passages):
    assert int(np.asarray(n_passages)) == NP
    nc = _get_nc()
    in_maps, t32, perms = prepare(q_reps, p_reps)
    res = run_bass_kernel_spmd(nc, in_maps, core_ids=list(range(NCORES)))
    return finalize(res.results, t32, perms)


def run_profiled(q_reps, p_reps, n_passages, mm_dt=None, trace=True):
    """Same as kernel() but returns (loss, BassKernelResults) with NTFF
    profile (requires the antenv.axon_hooks shim; see _install_ntff_shim)."""
    nc = _get_nc(mm_dt)
    in_maps, t32, perms = prepare(q_reps, p_reps, mm_dt)
    # warm-up / correctness run (also warms the PJRT executable cache)
    res = run_bass_kernel_spmd(nc, in_maps, core_ids=list(range(NCORES)),
                               trace=trace)
    loss = finalize(res.results, t32, perms)
    return loss, res


def _install_ntff_shim():
    """Provide antenv.axon_hooks (absent in this image) so trace=True works."""
    import types
    import antenv
    if "antenv.axon_hooks" in sys.modules:
        return
    mod = types.ModuleType("antenv.axon_hooks")
    mod._hook = None
    mod.set_axon_ntff_profile_hook = lambda h: setattr(mod, "_hook", h)
    mod.get_axon_ntff_profile_hook = lambda: mod._hook
    sys.modules["antenv.axon_hooks"] = mod
    antenv.axon_hooks = mod
    try:
        from trn_agent_boot.trn_boot import _ntff_profile_via_ctypes
        hook = _ntff_profile_via_ctypes("/opt/axon/libaxon_pjrt.so")
        if hook is not None:
            mod._hook = hook
    except Exception:
        pass


# revision 14
# speedup vs baseline: 1.2082x; 1.1257x over previous
"""Trainium2 Bass kernel for nn_DenseModel_51926154609008 (weighted-rank
contrastive CE loss).

Math (reference semantics, no sort needed):
  scores = q @ p.T                       [B=2048, P=16384]
  t_i    = scores[i, 8*i]                (positive/target score)
  rank_i = #{j : scores[i, j] > t_i}     (argsort position == exceed count,
                                          ties are measure-zero for randn data)
  lse_i  = logsumexp(scores[i, :])
  loss   = mean((lse_i - t_i) * (1 + 2.6*exp(-(rank_i-1)^2 / (2*1.8^2))))

Sharding: passage-parallel (P split across 8 cores, q replicated) — 12.6MB
of HBM reads per core vs 51MB for query-parallel with replicated passages.
Each core computes a [2048, 2048] score slab in 32 half-tiles
([128 queries x 1024 passages], one 2-bank PSUM buffer each) and reduces
every half-tile to per-query partials:
  sumexp_c[i] = sum_j exp(s_ij - C)      (fixed shift C so partials add
                                          across cores without a max-merge)
  cnt_c[i]    = #{j in slab : s_ij > t_i}
The host combines partials and evaluates the tiny [2048] tail in fp64.

The self-comparison (j == 8i) must contribute exactly 0 to rank_i. Query i's
target column lives only in core (i//256)'s slab. Each core rotates its query
order (data-level permutation — the program stays SPMD-uniform) so its own
queries always land on m-tiles OWN_M, OWN_M+1; the two half-tiles containing
self-columns use a masked count (indicator * mask, one fused DVE op); all
other half-tiles use a plain per-partition is_gt count.

t itself is computed on the host (trivial 2048x768 row-dot).

HW notes baked in from trace/bisect evidence:
  - DVE ops fault when an access pattern spans >2 PSUM banks; 2 banks is
    fine -> [128, 1024] half-tiles, one count op each.
  - ACT reads spanning 4 banks are fine; exp uses the per-instruction
    accumulator (sum along free dim) so no junk reduction is needed.
  - Mixing ACT functions (Exp/Sigmoid) forces ~1.3us ACT_TABLE_LOADs; the
    kernel uses Exp only.
  - bf16 matmuls stream at ~216ns per [128x512] MM warm; fp32 runs 2x
    slower and float32r ~1.9x (fp32_mode=HIGH, no FWL weight loads).
  - Input DMAs are split into [128, 512] sub-chunks, ordered so the first
    m-tile's operands land first (whole-tile DMAs starved the PE for ~14us).
"""

import sys

import numpy as np

sys.path.insert(0, "/opt/trn_rl_repo")

import concourse.bacc as bacc  # noqa: E402
import concourse.bass as bass  # noqa: E402
import concourse.mybir as mybir  # noqa: E402
import concourse.tile as tile  # noqa: E402
from concourse.bass_utils import run_bass_kernel_spmd  # noqa: E402

# Problem shape (hardcoded per the task contract).
B = 2048
D = 768
NP = 8
P = B * NP  # 16384
NCORES = 8
PSLAB = P // NCORES  # 2048 passage columns per core
KCH = D // 128  # 6 contraction chunks
MT = B // 128  # 16 query m-tiles
NU = 2 * MT  # 32 half-tile units of [128, 1024]
QSLAB = B // NCORES  # 256 queries owned per core
OWN_M = 8  # own queries sit at m-tiles 8,9 (mask off the critical path)

C_SHIFT = 128.0  # fixed exp shift: exp(s - C) never overflows for this data

ALPHA = 2.6
OPTIMAL_RANK = 1.0
SIGMA = 1.8

# Matmul input dtype: bfloat16 | float32r | float32
MM_DT = mybir.dt.bfloat16

_STATE: dict = {}


def _build_nc(mm_dt):
    nc = bacc.Bacc("TRN2", target_bir_lowering=False, debug=False,
                   num_devices=NCORES)

    qT_d = nc.dram_tensor("qT", [D, B], mm_dt, kind="ExternalInput").ap()
    pT_d = nc.dram_tensor("pT", [D, PSLAB], mm_dt, kind="ExternalInput").ap()
    tv_d = nc.dram_tensor("tvec", [128, MT], mybir.dt.float32,
                          kind="ExternalInput").ap()
    msk_d = nc.dram_tensor("msk", [128, 1024], mybir.dt.float32,
                           kind="ExternalInput").ap()
    se_d = nc.dram_tensor("se_out", [128, NU], mybir.dt.float32,
                          kind="ExternalOutput").ap()
    cnt_d = nc.dram_tensor("cnt_out", [128, NU], mybir.dt.float32,
                           kind="ExternalOutput").ap()

    f32 = mybir.dt.float32
    bf16 = mybir.dt.bfloat16

    with tile.TileContext(nc) as tc:
        with (
            tc.tile_pool(name="weights", bufs=1) as wpool,
            tc.tile_pool(name="stats", bufs=1) as spool,
            tc.tile_pool(name="junk", bufs=3) as jpool,
            tc.tile_pool(name="psum", bufs=4,
                         space=bass.MemorySpace.PSUM) as ppool,
        ):
            qk = [wpool.tile([128, B], mm_dt, name=f"qk{k}", tag=f"qk{k}")
                  for k in range(KCH)]
            pk = [wpool.tile([128, PSLAB], mm_dt, name=f"pk{k}", tag=f"pk{k}")
                  for k in range(KCH)]

            def ldq(k, part):
                nc.sync.dma_start(
                    qk[k][:, part * 512:(part + 1) * 512],
                    qT_d[k * 128:(k + 1) * 128, part * 512:(part + 1) * 512])

            def ldp(k, part):
                nc.sync.dma_start(
                    pk[k][:, part * 512:(part + 1) * 512],
                    pT_d[k * 128:(k + 1) * 128, part * 512:(part + 1) * 512])

            # first half-tile needs qk*.part0 + pk*.part0/1; order the
            # sub-chunk DMAs so those land first.
            for k in range(KCH):
                ldq(k, 0)
                ldp(k, 0)
            for k in range(KCH):
                ldp(k, 1)
            for k in range(KCH):
                ldp(k, 2)
                ldp(k, 3)
            tv = spool.tile([128, MT], f32, name="tv", tag="tv")
            nc.sync.dma_start(tv[:], tv_d[:])
            for part in range(1, 4):
                for k in range(KCH):
                    ldq(k, part)
            msk = spool.tile([128, 1024], f32, name="msk", tag="msk")

            se_sb = spool.tile([128, NU], f32, name="se_sb", tag="se_sb")
            cnt_sb = spool.tile([128, NU], f32, name="cnt_sb", tag="cnt_sb")
            negc = spool.tile([128, 1], f32, name="negc", tag="negc")
            nc.vector.memset(negc[:], -C_SHIFT)

            for u in range(NU):
                m, nh = u // 2, u % 2
                if u == 2 * OWN_M - 2:
                    # mask only needed by units 2*OWN_M and 2*OWN_M+3;
                    # fetch it shortly before, off the startup path.
                    nc.sync.dma_start(msk[:], msk_d[:])
                ps = ppool.tile([128, 1024], f32, name="ps", tag="ps")
                for nloc in range(2):
                    nb = nh * 2 + nloc
                    for k in range(KCH):
                        nc.tensor.matmul(
                            ps[:, nloc * 512:(nloc + 1) * 512],
                            qk[k][:, m * 128:(m + 1) * 128],
                            pk[k][:, nb * 512:(nb + 1) * 512],
                            start=(k == 0),
                            stop=(k == KCH - 1),
                        )
                je = jpool.tile([128, 1024], bf16, name="je", tag="je")
                nc.scalar.activation(
                    je[:], ps[:], mybir.ActivationFunctionType.Exp,
                    bias=negc[:], scale=1.0,
                    accum_out=se_sb[:, u:u + 1],
                )
                jc = jpool.tile([128, 1024], bf16, name="jc", tag="jc")
                if u in (2 * OWN_M, 2 * OWN_M + 3):
                    # half-tiles holding the self column: masked count
                    nc.vector.scalar_tensor_tensor(
                        out=jc[:], in0=ps[:], scalar=tv[:, m:m + 1],
                        in1=msk[:],
                        op0=mybir.AluOpType.is_gt,
                        op1=mybir.AluOpType.mult,
                        accum_out=cnt_sb[:, u:u + 1],
                    )
                else:
                    nc.vector.tensor_scalar(
                        jc[:], ps[:], tv[:, m:m + 1], None,
                        op0=mybir.AluOpType.is_gt,
                        op1=mybir.AluOpType.add,
                        accum_out=cnt_sb[:, u:u + 1],
                    )

            nc.sync.dma_start(se_d[:], se_sb[:])
            nc.sync.dma_start(cnt_d[:], cnt_sb[:])

    nc.compile()
    return nc


def _np_dtype(mm_dt):
    if mm_dt == mybir.dt.bfloat16:
        import ml_dtypes
        return ml_dtypes.bfloat16
    return np.float32


def _perm(c):
    """Rotation putting core c's own queries at m-tiles OWN_M, OWN_M+1."""
    return np.roll(np.arange(B), OWN_M * 128 - c * QSLAB)


def prepare(q, p, mm_dt=None):
    """Host-side shard prep. Returns (in_maps, t32, perms)."""
    if mm_dt is None:
        mm_dt = MM_DT
    npdt = _np_dtype(mm_dt)
    q = np.ascontiguousarray(np.asarray(q, dtype=np.float32))
    p = np.ascontiguousarray(np.asarray(p, dtype=np.float32))

    # target scores t_i = q_i . p_{8i} (fp32; matches the reference's fp32
    # value to ~1e-7 — only a compare threshold + host-tail term)
    t32 = np.einsum("ij,ij->i", q, p[::NP], dtype=np.float64).astype(np.float32)

    qT = np.ascontiguousarray(q.T)  # [D, B] fp32
    r = np.arange(128)
    # self columns: unit 2*OWN_M has query pi=OWN_M*128+r vs local col 8r
    # (half 0); unit 2*OWN_M+3 has pi=(OWN_M+1)*128+r vs col 1024+8r
    # (i.e. col 8r of half 1). Same mask for both, same for every core.
    msk = np.ones((128, 1024), dtype=np.float32)
    msk[r, 8 * r] = 0.0

    in_maps = []
    perms = []
    for c in range(NCORES):
        perm = _perm(c)
        perms.append(perm)
        qTc = np.ascontiguousarray(qT[:, perm]).astype(npdt)
        pTc = np.ascontiguousarray(p[c * PSLAB:(c + 1) * PSLAB].T).astype(npdt)
        tvc = np.ascontiguousarray(t32[perm].reshape(MT, 128).T)
        in_maps.append({"qT": qTc, "pT": pTc, "tvec": tvc, "msk": msk})
    return in_maps, t32, perms


def finalize(results, t32, perms):
    """Combine per-core partials into the scalar loss (fp64 host tail)."""
    se_tot = np.zeros(B, dtype=np.float64)
    cnt_tot = np.zeros(B, dtype=np.float64)
    for c in range(NCORES):
        perm = perms[c]
        # column u = m*2 + nh; query pi = m*128 + r
        se = results[c]["se_out"].astype(np.float64)
        cnt = results[c]["cnt_out"].astype(np.float64)
        se_q = (se[:, 0::2] + se[:, 1::2]).T.ravel()
        cnt_q = (cnt[:, 0::2] + cnt[:, 1::2]).T.ravel()
        se_tot[perm] += se_q
        cnt_tot[perm] += cnt_q
    lse = C_SHIFT + np.log(se_tot)
    raw = lse - t32.astype(np.float64)
    w = 1.0 + ALPHA * np.exp(-((cnt_tot - OPTIMAL_RANK) ** 2)
                             / (2.0 * SIGMA ** 2))
    return np.float32(np.mean(raw * w))


def _get_nc(mm_dt=None):
    if mm_dt is None:
        mm_dt = MM_DT
    if mm_dt not in _STATE:
        _STATE[mm_dt] = _build_nc(mm_dt)
    return _STATE[mm_dt]


def kernel(q_reps, p_reps, n_passages):
    assert int(np.asarray(n_passages)) == NP
    nc = _get_nc()
    in_maps, t32, perms = prepare(q_reps, p_reps)
    res = run_bass_kernel_spmd(nc, in_maps, core_ids=list(range(NCORES)))
    return finalize(res.results, t32, perms)


def run_profiled(q_reps, p_reps, n_passages, mm_dt=None, trace=True):
    """Same as kernel() but returns (loss, BassKernelResults) with NTFF
    profile (requires the antenv.axon_hooks shim; see _install_ntff_shim)."""
    nc = _get_nc(mm_dt)
    in_maps, t32, perms = prepare(q_reps, p_reps, mm_dt)
    res = run_bass_kernel_spmd(nc, in_maps, core_ids=list(range(NCORES)),
                               trace=trace)
    loss = finalize(res.results, t32, perms)
    return loss, res


def _install_ntff_shim():
    """Provide antenv.axon_hooks (absent in this image) so trace=True works."""
    import types
    import antenv
    if "antenv.axon_hooks" in sys.modules:
        return
    mod = types.ModuleType("antenv.axon_hooks")
    mod._hook = None
    mod.set_axon_ntff_profile_hook = lambda h: setattr(mod, "_hook", h)
    mod.get_axon_ntff_profile_hook = lambda: mod._hook
    sys.modules["antenv.axon_hooks"] = mod
    antenv.axon_hooks = mod
    try:
        from trn_agent_boot.trn_boot import _ntff_profile_via_ctypes
        hook = _ntff_profile_via_ctypes("/opt/axon/libaxon_pjrt.so")
        if hook is not None:
            mod._hook = hook
    except Exception:
        pass


# revision 17
# speedup vs baseline: 1.3089x; 1.0834x over previous
"""Trainium2 Bass kernel for nn_DenseModel_51926154609008 (weighted-rank
contrastive CE loss).

Math (reference semantics, no sort needed):
  scores = q @ p.T                       [B=2048, P=16384]
  t_i    = scores[i, 8*i]                (positive/target score)
  rank_i = #{j : scores[i, j] > t_i}     (argsort position == exceed count,
                                          ties are measure-zero for randn data)
  lse_i  = logsumexp(scores[i, :])
  loss   = mean((lse_i - t_i) * (1 + 2.6*exp(-(rank_i-1)^2 / (2*1.8^2))))

Sharding: passage-parallel (P split across 8 cores, q replicated) — 12.6MB
of HBM reads per core vs 51MB for query-parallel with replicated passages.
Each core computes a [2048, 2048] score slab in 32 half-tiles
([128 queries x 1024 passages], one 2-bank PSUM buffer each) and reduces
every half-tile to per-query partials:
  sumexp_c[i] = sum_j exp(s_ij - C)      (fixed shift C so partials add
                                          across cores without a max-merge)
  cnt_c[i]    = #{j in slab : s_ij > t_i}
The host combines partials and evaluates the tiny [2048] tail in fp64.

The self-comparison (j == 8i) must contribute exactly 0 to rank_i. Query i's
target column lives only in core (i//256)'s slab. Each core rotates its query
order (data-level permutation — the program stays SPMD-uniform) so its own
queries always land on m-tiles OWN_M, OWN_M+1; the two half-tiles containing
self-columns use a masked count (indicator * mask, one fused DVE op); all
other half-tiles use a plain per-partition is_gt count.

t itself is computed on the host (trivial 2048x768 row-dot).

HW notes baked in from trace/bisect evidence:
  - DVE ops fault when an access pattern spans >2 PSUM banks; 2 banks is
    fine -> [128, 1024] half-tiles, one count op each.
  - ACT reads spanning 4 banks are fine; exp uses the per-instruction
    accumulator (sum along free dim) so no junk reduction is needed.
  - Mixing ACT functions (Exp/Sigmoid) forces ~1.3us ACT_TABLE_LOADs; the
    kernel uses Exp only.
  - bf16 matmuls stream at ~216ns per [128x512] MM warm; fp32 runs 2x
    slower and float32r ~1.9x (fp32_mode=HIGH, no FWL weight loads).
  - Input DMAs are split into [128, 512] sub-chunks, ordered so the first
    m-tile's operands land first (whole-tile DMAs starved the PE for ~14us).
"""

import sys

import numpy as np

sys.path.insert(0, "/opt/trn_rl_repo")

import concourse.bacc as bacc  # noqa: E402
import concourse.bass as bass  # noqa: E402
import concourse.mybir as mybir  # noqa: E402
import concourse.tile as tile  # noqa: E402
from concourse.bass_utils import run_bass_kernel_spmd  # noqa: E402

# Problem shape (hardcoded per the task contract).
B = 2048
D = 768
NP = 8
P = B * NP  # 16384
NCORES = 8
PSLAB = P // NCORES  # 2048 passage columns per core
KCH = D // 128  # 6 contraction chunks
MT = B // 128  # 16 query m-tiles
NU = 2 * MT  # 32 half-tile units of [128, 1024]
QSLAB = B // NCORES  # 256 queries owned per core
OWN_M = 8  # own queries sit at m-tiles 8,9 (mask off the critical path)

C_SHIFT = 128.0  # fixed exp shift: exp(s - C) never overflows for this data

ALPHA = 2.6
OPTIMAL_RANK = 1.0
SIGMA = 1.8

# Matmul input dtype: bfloat16 | float32r | float32
MM_DT = mybir.dt.bfloat16

_STATE: dict = {}


def _build_nc(mm_dt):
    nc = bacc.Bacc("TRN2", target_bir_lowering=False, debug=False,
                   num_devices=NCORES)

    qT_d = nc.dram_tensor("qT", [D, B], mm_dt, kind="ExternalInput").ap()
    pT_d = nc.dram_tensor("pT", [D, PSLAB], mm_dt, kind="ExternalInput").ap()
    tv_d = nc.dram_tensor("tvec", [128, MT], mybir.dt.float32,
                          kind="ExternalInput").ap()
    msk_d = nc.dram_tensor("msk", [128, 1024], mybir.dt.float32,
                           kind="ExternalInput").ap()
    se_d = nc.dram_tensor("se_out", [128, NU], mybir.dt.float32,
                          kind="ExternalOutput").ap()
    cnt_d = nc.dram_tensor("cnt_out", [128, NU], mybir.dt.float32,
                           kind="ExternalOutput").ap()

    f32 = mybir.dt.float32
    bf16 = mybir.dt.bfloat16

    with tile.TileContext(nc) as tc:
        with (
            tc.tile_pool(name="weights", bufs=1) as wpool,
            tc.tile_pool(name="stats", bufs=1) as spool,
            tc.tile_pool(name="junk", bufs=3) as jpool,
            tc.tile_pool(name="psum", bufs=4,
                         space=bass.MemorySpace.PSUM) as ppool,
        ):
            qk = [wpool.tile([128, B], mm_dt, name=f"qk{k}", tag=f"qk{k}")
                  for k in range(KCH)]
            pk = [wpool.tile([128, PSLAB], mm_dt, name=f"pk{k}", tag=f"pk{k}")
                  for k in range(KCH)]

            def ldq(k, part):  # issued on GpSimd's sequencer
                nc.gpsimd.dma_start(
                    qk[k][:, part * 512:(part + 1) * 512],
                    qT_d[k * 128:(k + 1) * 128, part * 512:(part + 1) * 512])

            def ldp(k, half):  # issued on Sync's sequencer
                nc.sync.dma_start(
                    pk[k][:, half * 1024:(half + 1) * 1024],
                    pT_d[k * 128:(k + 1) * 128, half * 1024:(half + 1) * 1024])

            # Units run nh-major (all half-0 m-tiles, then all half-1), so
            # pk half 1 isn't needed until mid-kernel; qk part p feeds
            # m-tiles 4p..4p+3. DMA issue is ~0.6us per dma_start on the
            # issuing sequencer, so the critical first operands go first,
            # split across two sequencers (Sync: pk, GpSimd: qk).
            tv = spool.tile([128, MT], f32, name="tv", tag="tv")
            msk = spool.tile([128, 1024], f32, name="msk", tag="msk")
            nc.sync.dma_start(tv[:], tv_d[:])
            for k in range(KCH):
                ldq(k, 0)
                ldp(k, 0)
            for k in range(KCH):
                ldq(k, 1)
            nc.gpsimd.dma_start(msk[:], msk_d[:])
            for k in range(KCH):
                ldp(k, 1)
                ldq(k, 2)
            for k in range(KCH):
                ldq(k, 3)

            se_sb = spool.tile([128, NU], f32, name="se_sb", tag="se_sb")
            cnt_sb = spool.tile([128, NU], f32, name="cnt_sb", tag="cnt_sb")
            negc = spool.tile([128, 1], f32, name="negc", tag="negc")
            nc.vector.memset(negc[:], -C_SHIFT)

            for u in range(NU):
                nh, m = u // MT, u % MT
                ps = ppool.tile([128, 1024], f32, name="ps", tag="ps")
                for nloc in range(2):
                    nb = nh * 2 + nloc
                    for k in range(KCH):
                        nc.tensor.matmul(
                            ps[:, nloc * 512:(nloc + 1) * 512],
                            qk[k][:, m * 128:(m + 1) * 128],
                            pk[k][:, nb * 512:(nb + 1) * 512],
                            start=(k == 0),
                            stop=(k == KCH - 1),
                        )
                je = jpool.tile([128, 1024], bf16, name="je", tag="je")
                nc.scalar.activation(
                    je[:], ps[:], mybir.ActivationFunctionType.Exp,
                    bias=negc[:], scale=1.0,
                    accum_out=se_sb[:, u:u + 1],
                )
                jc = jpool.tile([128, 1024], bf16, name="jc", tag="jc")
                if u in (OWN_M, MT + OWN_M + 1):
                    # half-tiles holding the self column: masked count
                    nc.vector.scalar_tensor_tensor(
                        out=jc[:], in0=ps[:], scalar=tv[:, m:m + 1],
                        in1=msk[:],
                        op0=mybir.AluOpType.is_gt,
                        op1=mybir.AluOpType.mult,
                        accum_out=cnt_sb[:, u:u + 1],
                    )
                else:
                    nc.vector.tensor_scalar(
                        jc[:], ps[:], tv[:, m:m + 1], None,
                        op0=mybir.AluOpType.is_gt,
                        op1=mybir.AluOpType.add,
                        accum_out=cnt_sb[:, u:u + 1],
                    )

            nc.sync.dma_start(se_d[:], se_sb[:])
            nc.sync.dma_start(cnt_d[:], cnt_sb[:])

    nc.compile()
    return nc


def _np_dtype(mm_dt):
    if mm_dt == mybir.dt.bfloat16:
        import ml_dtypes
        return ml_dtypes.bfloat16
    return np.float32


def _perm(c):
    """Rotation putting core c's own queries at m-tiles OWN_M, OWN_M+1."""
    return np.roll(np.arange(B), OWN_M * 128 - c * QSLAB)


def prepare(q, p, mm_dt=None):
    """Host-side shard prep. Returns (in_maps, t32, perms)."""
    if mm_dt is None:
        mm_dt = MM_DT
    npdt = _np_dtype(mm_dt)
    q = np.ascontiguousarray(np.asarray(q, dtype=np.float32))
    p = np.ascontiguousarray(np.asarray(p, dtype=np.float32))

    # target scores t_i = q_i . p_{8i} (fp32; matches the reference's fp32
    # value to ~1e-7 — only a compare threshold + host-tail term)
    t32 = np.einsum("ij,ij->i", q, p[::NP], dtype=np.float64).astype(np.float32)

    qT = np.ascontiguousarray(q.T)  # [D, B] fp32
    r = np.arange(128)
    # self columns: unit 2*OWN_M has query pi=OWN_M*128+r vs local col 8r
    # (half 0); unit 2*OWN_M+3 has pi=(OWN_M+1)*128+r vs col 1024+8r
    # (i.e. col 8r of half 1). Same mask for both, same for every core.
    msk = np.ones((128, 1024), dtype=np.float32)
    msk[r, 8 * r] = 0.0

    in_maps = []
    perms = []
    for c in range(NCORES):
        perm = _perm(c)
        perms.append(perm)
        qTc = np.ascontiguousarray(qT[:, perm]).astype(npdt)
        pTc = np.ascontiguousarray(p[c * PSLAB:(c + 1) * PSLAB].T).astype(npdt)
        tvc = np.ascontiguousarray(t32[perm].reshape(MT, 128).T)
        in_maps.append({"qT": qTc, "pT": pTc, "tvec": tvc, "msk": msk})
    return in_maps, t32, perms


def finalize(results, t32, perms):
    """Combine per-core partials into the scalar loss (fp64 host tail)."""
    se_tot = np.zeros(B, dtype=np.float64)
    cnt_tot = np.zeros(B, dtype=np.float64)
    for c in range(NCORES):
        perm = perms[c]
        # column u = nh*MT + m; query pi = m*128 + r
        se = results[c]["se_out"].astype(np.float64)
        cnt = results[c]["cnt_out"].astype(np.float64)
        se_q = (se[:, :MT] + se[:, MT:]).T.ravel()
        cnt_q = (cnt[:, :MT] + cnt[:, MT:]).T.ravel()
        se_tot[perm] += se_q
        cnt_tot[perm] += cnt_q
    lse = C_SHIFT + np.log(se_tot)
    raw = lse - t32.astype(np.float64)
    w = 1.0 + ALPHA * np.exp(-((cnt_tot - OPTIMAL_RANK) ** 2)
                             / (2.0 * SIGMA ** 2))
    return np.float32(np.mean(raw * w))


def _get_nc(mm_dt=None):
    if mm_dt is None:
        mm_dt = MM_DT
    if mm_dt not in _STATE:
        _STATE[mm_dt] = _build_nc(mm_dt)
    return _STATE[mm_dt]


def kernel(q_reps, p_reps, n_passages):
    assert int(np.asarray(n_passages)) == NP
    nc = _get_nc()
    in_maps, t32, perms = prepare(q_reps, p_reps)
    res = run_bass_kernel_spmd(nc, in_maps, core_ids=list(range(NCORES)))
    return finalize(res.results, t32, perms)


def run_profiled(q_reps, p_reps, n_passages, mm_dt=None, trace=True):
    """Same as kernel() but returns (loss, BassKernelResults) with NTFF
    profile (requires the antenv.axon_hooks shim; see _install_ntff_shim)."""
    nc = _get_nc(mm_dt)
    in_maps, t32, perms = prepare(q_reps, p_reps, mm_dt)
    res = run_bass_kernel_spmd(nc, in_maps, core_ids=list(range(NCORES)),
                               trace=trace)
    loss = finalize(res.results, t32, perms)
    return loss, res


def _install_ntff_shim():
    """Provide antenv.axon_hooks (absent in this image) so trace=True works."""
    import types
    import antenv
    if "antenv.axon_hooks" in sys.modules:
        return
    mod = types.ModuleType("antenv.axon_hooks")
    mod._hook = None
    mod.set_axon_ntff_profile_hook = lambda h: setattr(mod, "_hook", h)
    mod.get_axon_ntff_profile_hook = lambda: mod._hook
    sys.modules["antenv.axon_hooks"] = mod
    antenv.axon_hooks = mod
    try:
        from trn_agent_boot.trn_boot import _ntff_profile_via_ctypes
        hook = _ntff_profile_via_ctypes("/opt/axon/libaxon_pjrt.so")
        if hook is not None:
            mod._hook = hook
    except Exception:
        pass


# revision 18
# speedup vs baseline: 1.3261x; 1.0131x over previous
"""Trainium2 Bass kernel for nn_DenseModel_51926154609008 (weighted-rank
contrastive CE loss).

Math (reference semantics, no sort needed):
  scores = q @ p.T                       [B=2048, P=16384]
  t_i    = scores[i, 8*i]                (positive/target score)
  rank_i = #{j : scores[i, j] > t_i}     (argsort position == exceed count,
                                          ties are measure-zero for randn data)
  lse_i  = logsumexp(scores[i, :])
  loss   = mean((lse_i - t_i) * (1 + 2.6*exp(-(rank_i-1)^2 / (2*1.8^2))))

Sharding: passage-parallel (P split across 8 cores, q replicated) — 12.6MB
of HBM reads per core vs 51MB for query-parallel with replicated passages.
Each core computes a [2048, 2048] score slab in 32 half-tiles
([128 queries x 1024 passages], one 2-bank PSUM buffer each) and reduces
every half-tile to per-query partials:
  sumexp_c[i] = sum_j exp(s_ij - C)      (fixed shift C so partials add
                                          across cores without a max-merge)
  cnt_c[i]    = #{j in slab : s_ij > t_i}
The host combines partials and evaluates the tiny [2048] tail in fp64.

The self-comparison (j == 8i) must contribute exactly 0 to rank_i. Query i's
target column lives only in core (i//256)'s slab. Each core rotates its query
order (data-level permutation — the program stays SPMD-uniform) so its own
queries always land on m-tiles OWN_M, OWN_M+1; the two half-tiles containing
self-columns use a masked count (indicator * mask, one fused DVE op); all
other half-tiles use a plain per-partition is_gt count.

t itself is computed on the host (trivial 2048x768 row-dot).

HW notes baked in from trace/bisect evidence:
  - DVE ops fault when an access pattern spans >2 PSUM banks; 2 banks is
    fine -> [128, 1024] half-tiles, one count op each.
  - ACT reads spanning 4 banks are fine; exp uses the per-instruction
    accumulator (sum along free dim) so no junk reduction is needed.
  - Mixing ACT functions (Exp/Sigmoid) forces ~1.3us ACT_TABLE_LOADs; the
    kernel uses Exp only.
  - bf16 matmuls stream at ~216ns per [128x512] MM warm; fp32 runs 2x
    slower and float32r ~1.9x (fp32_mode=HIGH, no FWL weight loads).
  - Input DMAs are split into [128, 512] sub-chunks, ordered so the first
    m-tile's operands land first (whole-tile DMAs starved the PE for ~14us).
"""

import sys

import numpy as np

sys.path.insert(0, "/opt/trn_rl_repo")

import concourse.bacc as bacc  # noqa: E402
import concourse.bass as bass  # noqa: E402
import concourse.mybir as mybir  # noqa: E402
import concourse.tile as tile  # noqa: E402
from concourse.bass_utils import run_bass_kernel_spmd  # noqa: E402

# Problem shape (hardcoded per the task contract).
B = 2048
D = 768
NP = 8
P = B * NP  # 16384
NCORES = 8
PSLAB = P // NCORES  # 2048 passage columns per core
KCH = D // 128  # 6 contraction chunks
MT = B // 128  # 16 query m-tiles
NU = 2 * MT  # 32 half-tile units of [128, 1024]
QSLAB = B // NCORES  # 256 queries owned per core
OWN_M = 8  # own queries sit at m-tiles 8,9 (mask off the critical path)

C_SHIFT = 128.0  # fixed exp shift: exp(s - C) never overflows for this data

ALPHA = 2.6
OPTIMAL_RANK = 1.0
SIGMA = 1.8

# Matmul input dtype: bfloat16 | float32r | float32
MM_DT = mybir.dt.bfloat16

_STATE: dict = {}


def _build_nc(mm_dt):
    nc = bacc.Bacc("TRN2", target_bir_lowering=False, debug=False,
                   num_devices=NCORES)

    qT_d = nc.dram_tensor("qT", [D, B], mm_dt, kind="ExternalInput").ap()
    pT_d = nc.dram_tensor("pT", [D, PSLAB], mm_dt, kind="ExternalInput").ap()
    tv_d = nc.dram_tensor("tvec", [128, MT], mybir.dt.float32,
                          kind="ExternalInput").ap()
    msk_d = nc.dram_tensor("msk", [128, 1024], mybir.dt.float32,
                           kind="ExternalInput").ap()
    se_d = nc.dram_tensor("se_out", [128, NU], mybir.dt.float32,
                          kind="ExternalOutput").ap()
    cnt_d = nc.dram_tensor("cnt_out", [128, NU], mybir.dt.float32,
                           kind="ExternalOutput").ap()

    f32 = mybir.dt.float32
    bf16 = mybir.dt.bfloat16

    with tile.TileContext(nc) as tc:
        with (
            tc.tile_pool(name="weights", bufs=1) as wpool,
            tc.tile_pool(name="stats", bufs=1) as spool,
            tc.tile_pool(name="junk", bufs=3) as jpool,
            tc.tile_pool(name="psum", bufs=4,
                         space=bass.MemorySpace.PSUM) as ppool,
        ):
            qk = [wpool.tile([128, B], mm_dt, name=f"qk{k}", tag=f"qk{k}")
                  for k in range(KCH)]
            pk = [wpool.tile([128, PSLAB], mm_dt, name=f"pk{k}", tag=f"pk{k}")
                  for k in range(KCH)]

            def ldq(k, part):  # issued on GpSimd's sequencer
                nc.gpsimd.dma_start(
                    qk[k][:, part * 512:(part + 1) * 512],
                    qT_d[k * 128:(k + 1) * 128, part * 512:(part + 1) * 512])

            def ldp(k, half):  # issued on Sync's sequencer
                nc.sync.dma_start(
                    pk[k][:, half * 1024:(half + 1) * 1024],
                    pT_d[k * 128:(k + 1) * 128, half * 1024:(half + 1) * 1024])

            # Units run nh-major (all half-0 m-tiles, then all half-1), so
            # pk half 1 isn't needed until mid-kernel; qk part p feeds
            # m-tiles 4p..4p+3. DMA issue is ~0.6us per dma_start on the
            # issuing sequencer, so the critical first operands go first,
            # split across two sequencers (Sync: pk, GpSimd: qk).
            tv = spool.tile([128, MT], f32, name="tv", tag="tv")
            msk = spool.tile([128, 1024], f32, name="msk", tag="msk")
            # smallest-possible first dependencies: MM#0 needs qk0 cols
            # 0:128 (LDWEIGHTS) and pk0 cols 0:512 only
            nc.gpsimd.dma_start(qk[0][:, 0:128], qT_d[0:128, 0:128])
            nc.sync.dma_start(pk[0][:, 0:512], pT_d[0:128, 0:512])
            nc.gpsimd.dma_start(qk[0][:, 128:512], qT_d[0:128, 128:512])
            nc.sync.dma_start(pk[0][:, 512:1024], pT_d[0:128, 512:1024])
            for k in range(1, KCH):
                ldq(k, 0)
                ldp(k, 0)
            nc.sync.dma_start(tv[:], tv_d[:])
            for k in range(KCH):
                ldq(k, 1)
            nc.gpsimd.dma_start(msk[:], msk_d[:])
            for k in range(KCH):
                ldp(k, 1)
                ldq(k, 2)
            for k in range(KCH):
                ldq(k, 3)

            se_sb = spool.tile([128, NU], f32, name="se_sb", tag="se_sb")
            cnt_sb = spool.tile([128, NU], f32, name="cnt_sb", tag="cnt_sb")
            negc = spool.tile([128, 1], f32, name="negc", tag="negc")
            nc.vector.memset(negc[:], -C_SHIFT)

            for u in range(NU):
                nh, m = u // MT, u % MT
                ps = ppool.tile([128, 1024], f32, name="ps", tag="ps")
                for nloc in range(2):
                    nb = nh * 2 + nloc
                    for k in range(KCH):
                        nc.tensor.matmul(
                            ps[:, nloc * 512:(nloc + 1) * 512],
                            qk[k][:, m * 128:(m + 1) * 128],
                            pk[k][:, nb * 512:(nb + 1) * 512],
                            start=(k == 0),
                            stop=(k == KCH - 1),
                        )
                je = jpool.tile([128, 1024], bf16, name="je", tag="je")
                nc.scalar.activation(
                    je[:], ps[:], mybir.ActivationFunctionType.Exp,
                    bias=negc[:], scale=1.0,
                    accum_out=se_sb[:, u:u + 1],
                )
                jc = jpool.tile([128, 1024], bf16, name="jc", tag="jc")
                if u in (OWN_M, MT + OWN_M + 1):
                    # half-tiles holding the self column: masked count
                    nc.vector.scalar_tensor_tensor(
                        out=jc[:], in0=ps[:], scalar=tv[:, m:m + 1],
                        in1=msk[:],
                        op0=mybir.AluOpType.is_gt,
                        op1=mybir.AluOpType.mult,
                        accum_out=cnt_sb[:, u:u + 1],
                    )
                else:
                    nc.vector.tensor_scalar(
                        jc[:], ps[:], tv[:, m:m + 1], None,
                        op0=mybir.AluOpType.is_gt,
                        op1=mybir.AluOpType.add,
                        accum_out=cnt_sb[:, u:u + 1],
                    )

            nc.sync.dma_start(se_d[:], se_sb[:])
            nc.sync.dma_start(cnt_d[:], cnt_sb[:])

    nc.compile()
    return nc


def _np_dtype(mm_dt):
    if mm_dt == mybir.dt.bfloat16:
        import ml_dtypes
        return ml_dtypes.bfloat16
    return np.float32


def _perm(c):
    """Rotation putting core c's own queries at m-tiles OWN_M, OWN_M+1."""
    return np.roll(np.arange(B), OWN_M * 128 - c * QSLAB)


def prepare(q, p, mm_dt=None):
    """Host-side shard prep. Returns (in_maps, t32, perms)."""
    if mm_dt is None:
        mm_dt = MM_DT
    npdt = _np_dtype(mm_dt)
    q = np.ascontiguousarray(np.asarray(q, dtype=np.float32))
    p = np.ascontiguousarray(np.asarray(p, dtype=np.float32))

    # target scores t_i = q_i . p_{8i} (fp32; matches the reference's fp32
    # value to ~1e-7 — only a compare threshold + host-tail term)
    t32 = np.einsum("ij,ij->i", q, p[::NP], dtype=np.float64).astype(np.float32)

    qT = np.ascontiguousarray(q.T)  # [D, B] fp32
    r = np.arange(128)
    # self columns: unit 2*OWN_M has query pi=OWN_M*128+r vs local col 8r
    # (half 0); unit 2*OWN_M+3 has pi=(OWN_M+1)*128+r vs col 1024+8r
    # (i.e. col 8r of half 1). Same mask for both, same for every core.
    msk = np.ones((128, 1024), dtype=np.float32)
    msk[r, 8 * r] = 0.0

    in_maps = []
    perms = []
    for c in range(NCORES):
        perm = _perm(c)
        perms.append(perm)
        qTc = np.ascontiguousarray(qT[:, perm]).astype(npdt)
        pTc = np.ascontiguousarray(p[c * PSLAB:(c + 1) * PSLAB].T).astype(npdt)
        tvc = np.ascontiguousarray(t32[perm].reshape(MT, 128).T)
        in_maps.append({"qT": qTc, "pT": pTc, "tvec": tvc, "msk": msk})
    return in_maps, t32, perms


def finalize(results, t32, perms):
    """Combine per-core partials into the scalar loss (fp64 host tail)."""
    se_tot = np.zeros(B, dtype=np.float64)
    cnt_tot = np.zeros(B, dtype=np.float64)
    for c in range(NCORES):
        perm = perms[c]
        # column u = nh*MT + m; query pi = m*128 + r
        se = results[c]["se_out"].astype(np.float64)
        cnt = results[c]["cnt_out"].astype(np.float64)
        se_q = (se[:, :MT] + se[:, MT:]).T.ravel()
        cnt_q = (cnt[:, :MT] + cnt[:, MT:]).T.ravel()
        se_tot[perm] += se_q
        cnt_tot[perm] += cnt_q
    lse = C_SHIFT + np.log(se_tot)
    raw = lse - t32.astype(np.float64)
    w = 1.0 + ALPHA * np.exp(-((cnt_tot - OPTIMAL_RANK) ** 2)
                             / (2.0 * SIGMA ** 2))
    return np.float32(np.mean(raw * w))


def _get_nc(mm_dt=None):
    if mm_dt is None:
        mm_dt = MM_DT
    if mm_dt not in _STATE:
        _STATE[mm_dt] = _build_nc(mm_dt)
    return _STATE[mm_dt]


def kernel(q_reps, p_reps, n_passages):
    assert int(np.asarray(n_passages)) == NP
    nc = _get_nc()
    in_maps, t32, perms = prepare(q_reps, p_reps)
    res = run_bass_kernel_spmd(nc, in_maps, core_ids=list(range(NCORES)))
    return finalize(res.results, t32, perms)


def run_profiled(q_reps, p_reps, n_passages, mm_dt=None, trace=True):
    """Same as kernel() but returns (loss, BassKernelResults) with NTFF
    profile (requires the antenv.axon_hooks shim; see _install_ntff_shim)."""
    nc = _get_nc(mm_dt)
    in_maps, t32, perms = prepare(q_reps, p_reps, mm_dt)
    res = run_bass_kernel_spmd(nc, in_maps, core_ids=list(range(NCORES)),
                               trace=trace)
    loss = finalize(res.results, t32, perms)
    return loss, res


def _install_ntff_shim():
    """Provide antenv.axon_hooks (absent in this image) so trace=True works."""
    import types
    import antenv
    if "antenv.axon_hooks" in sys.modules:
        return
    mod = types.ModuleType("antenv.axon_hooks")
    mod._hook = None
    mod.set_axon_ntff_profile_hook = lambda h: setattr(mod, "_hook", h)
    mod.get_axon_ntff_profile_hook = lambda: mod._hook
    sys.modules["antenv.axon_hooks"] = mod
    antenv.axon_hooks = mod
    try:
        from trn_agent_boot.trn_boot import _ntff_profile_via_ctypes
        hook = _ntff_profile_via_ctypes("/opt/axon/libaxon_pjrt.so")
        if hook is not None:
            mod._hook = hook
    except Exception:
        pass


# revision 19
# speedup vs baseline: 1.3297x; 1.0027x over previous
"""Trainium2 Bass kernel for nn_DenseModel_51926154609008 (weighted-rank
contrastive CE loss).

Math (reference semantics, no sort needed):
  scores = q @ p.T                       [B=2048, P=16384]
  t_i    = scores[i, 8*i]                (positive/target score)
  rank_i = #{j : scores[i, j] > t_i}     (argsort position == exceed count,
                                          ties are measure-zero for randn data)
  lse_i  = logsumexp(scores[i, :])
  loss   = mean((lse_i - t_i) * (1 + 2.6*exp(-(rank_i-1)^2 / (2*1.8^2))))

Sharding: passage-parallel (P split across 8 cores, q replicated) — 12.6MB
of HBM reads per core vs 51MB for query-parallel with replicated passages.
Each core computes a [2048, 2048] score slab in 32 half-tiles
([128 queries x 1024 passages], one 2-bank PSUM buffer each) and reduces
every half-tile to per-query partials:
  sumexp_c[i] = sum_j exp(s_ij - C)      (fixed shift C so partials add
                                          across cores without a max-merge)
  cnt_c[i]    = #{j in slab : s_ij > t_i}
The host combines partials and evaluates the tiny [2048] tail in fp64.

The self-comparison (j == 8i) must contribute exactly 0 to rank_i. Query i's
target column lives only in core (i//256)'s slab. Each core rotates its query
order (data-level permutation — the program stays SPMD-uniform) so its own
queries always land on m-tiles OWN_M, OWN_M+1; the two half-tiles containing
self-columns use a masked count (indicator * mask, one fused DVE op); all
other half-tiles use a plain per-partition is_gt count.

t itself is computed on the host (trivial 2048x768 row-dot).

HW notes baked in from trace/bisect evidence:
  - DVE ops fault when an access pattern spans >2 PSUM banks; 2 banks is
    fine -> [128, 1024] half-tiles, one count op each.
  - ACT reads spanning 4 banks are fine; exp uses the per-instruction
    accumulator (sum along free dim) so no junk reduction is needed.
  - Mixing ACT functions (Exp/Sigmoid) forces ~1.3us ACT_TABLE_LOADs; the
    kernel uses Exp only.
  - bf16 matmuls stream at ~216ns per [128x512] MM warm; fp32 runs 2x
    slower and float32r ~1.9x (fp32_mode=HIGH, no FWL weight loads).
  - Input DMAs are split into [128, 512] sub-chunks, ordered so the first
    m-tile's operands land first (whole-tile DMAs starved the PE for ~14us).
"""

import sys

import numpy as np

sys.path.insert(0, "/opt/trn_rl_repo")

import concourse.bacc as bacc  # noqa: E402
import concourse.bass as bass  # noqa: E402
import concourse.mybir as mybir  # noqa: E402
import concourse.tile as tile  # noqa: E402
from concourse.bass_utils import run_bass_kernel_spmd  # noqa: E402

# Problem shape (hardcoded per the task contract).
B = 2048
D = 768
NP = 8
P = B * NP  # 16384
NCORES = 8
PSLAB = P // NCORES  # 2048 passage columns per core
KCH = D // 128  # 6 contraction chunks
MT = B // 128  # 16 query m-tiles
NU = 2 * MT  # 32 half-tile units of [128, 1024]
QSLAB = B // NCORES  # 256 queries owned per core
OWN_M = 8  # own queries sit at m-tiles 8,9 (mask off the critical path)

C_SHIFT = 128.0  # fixed exp shift: exp(s - C) never overflows for this data

ALPHA = 2.6
OPTIMAL_RANK = 1.0
SIGMA = 1.8

# Matmul input dtype: bfloat16 | float32r | float32
MM_DT = mybir.dt.bfloat16

_STATE: dict = {}


def _build_nc(mm_dt):
    nc = bacc.Bacc("TRN2", target_bir_lowering=False, debug=False,
                   num_devices=NCORES)

    qT_d = nc.dram_tensor("qT", [D, B], mm_dt, kind="ExternalInput").ap()
    pT_d = nc.dram_tensor("pT", [D, PSLAB], mm_dt, kind="ExternalInput").ap()
    tv_d = nc.dram_tensor("tvec", [128, MT], mybir.dt.float32,
                          kind="ExternalInput").ap()
    msk_d = nc.dram_tensor("msk", [128, 1024], mybir.dt.float32,
                           kind="ExternalInput").ap()
    se_d = nc.dram_tensor("se_out", [128, NU], mybir.dt.float32,
                          kind="ExternalOutput").ap()
    cnt_d = nc.dram_tensor("cnt_out", [128, NU], mybir.dt.float32,
                           kind="ExternalOutput").ap()

    f32 = mybir.dt.float32
    bf16 = mybir.dt.bfloat16

    with tile.TileContext(nc) as tc:
        with (
            tc.tile_pool(name="weights", bufs=1) as wpool,
            tc.tile_pool(name="stats", bufs=1) as spool,
            tc.tile_pool(name="junk", bufs=3) as jpool,
            tc.tile_pool(name="psum", bufs=4,
                         space=bass.MemorySpace.PSUM) as ppool,
        ):
            qk = [wpool.tile([128, B], mm_dt, name=f"qk{k}", tag=f"qk{k}")
                  for k in range(KCH)]
            pk = [wpool.tile([128, PSLAB], mm_dt, name=f"pk{k}", tag=f"pk{k}")
                  for k in range(KCH)]

            def ldq(k, part):  # issued on GpSimd's sequencer
                nc.gpsimd.dma_start(
                    qk[k][:, part * 512:(part + 1) * 512],
                    qT_d[k * 128:(k + 1) * 128, part * 512:(part + 1) * 512])

            def ldp(k, half):  # issued on Sync's sequencer
                nc.sync.dma_start(
                    pk[k][:, half * 1024:(half + 1) * 1024],
                    pT_d[k * 128:(k + 1) * 128, half * 1024:(half + 1) * 1024])

            # Units run nh-major (all half-0 m-tiles, then all half-1), so
            # pk half 1 isn't needed until mid-kernel; qk part p feeds
            # m-tiles 4p..4p+3. DMA issue is ~0.6us per dma_start on the
            # issuing sequencer, so the critical first operands go first,
            # split across two sequencers (Sync: pk, GpSimd: qk).
            tv = spool.tile([128, MT], f32, name="tv", tag="tv")
            msk = spool.tile([128, 1024], f32, name="msk", tag="msk")
            # smallest-possible first dependencies: MM#0 needs qk0 cols
            # 0:128 (LDWEIGHTS) and pk0 cols 0:512 only
            nc.gpsimd.dma_start(qk[0][:, 0:128], qT_d[0:128, 0:128])
            nc.sync.dma_start(pk[0][:, 0:512], pT_d[0:128, 0:512])
            nc.gpsimd.dma_start(qk[0][:, 128:512], qT_d[0:128, 128:512])
            nc.sync.dma_start(pk[0][:, 512:1024], pT_d[0:128, 512:1024])
            for k in range(1, KCH):
                ldq(k, 0)
                if k <= 3:
                    nc.sync.dma_start(pk[k][:, 0:1024],
                                      pT_d[k * 128:(k + 1) * 128, 0:1024])
                else:
                    # third sequencer so the k-chain of first-unit operands
                    # arrives faster than the PE consumes it
                    nc.scalar.dma_start(pk[k][:, 0:1024],
                                        pT_d[k * 128:(k + 1) * 128, 0:1024])
            nc.sync.dma_start(tv[:], tv_d[:])
            for k in range(KCH):
                ldq(k, 1)
            nc.gpsimd.dma_start(msk[:], msk_d[:])
            for k in range(KCH):
                ldp(k, 1)
                ldq(k, 2)
            for k in range(KCH):
                ldq(k, 3)

            se_sb = spool.tile([128, NU], f32, name="se_sb", tag="se_sb")
            cnt_sb = spool.tile([128, NU], f32, name="cnt_sb", tag="cnt_sb")
            negc = spool.tile([128, 1], f32, name="negc", tag="negc")
            nc.vector.memset(negc[:], -C_SHIFT)

            for u in range(NU):
                nh, m = u // MT, u % MT
                ps = ppool.tile([128, 1024], f32, name="ps", tag="ps")
                for nloc in range(2):
                    nb = nh * 2 + nloc
                    for k in range(KCH):
                        nc.tensor.matmul(
                            ps[:, nloc * 512:(nloc + 1) * 512],
                            qk[k][:, m * 128:(m + 1) * 128],
                            pk[k][:, nb * 512:(nb + 1) * 512],
                            start=(k == 0),
                            stop=(k == KCH - 1),
                        )
                je = jpool.tile([128, 1024], bf16, name="je", tag="je")
                nc.scalar.activation(
                    je[:], ps[:], mybir.ActivationFunctionType.Exp,
                    bias=negc[:], scale=1.0,
                    accum_out=se_sb[:, u:u + 1],
                )
                jc = jpool.tile([128, 1024], bf16, name="jc", tag="jc")
                if u in (OWN_M, MT + OWN_M + 1):
                    # half-tiles holding the self column: masked count
                    nc.vector.scalar_tensor_tensor(
                        out=jc[:], in0=ps[:], scalar=tv[:, m:m + 1],
                        in1=msk[:],
                        op0=mybir.AluOpType.is_gt,
                        op1=mybir.AluOpType.mult,
                        accum_out=cnt_sb[:, u:u + 1],
                    )
                else:
                    nc.vector.tensor_scalar(
                        jc[:], ps[:], tv[:, m:m + 1], None,
                        op0=mybir.AluOpType.is_gt,
                        op1=mybir.AluOpType.add,
                        accum_out=cnt_sb[:, u:u + 1],
                    )

            nc.sync.dma_start(se_d[:], se_sb[:])
            nc.sync.dma_start(cnt_d[:], cnt_sb[:])

    nc.compile()
    return nc


def _np_dtype(mm_dt):
    if mm_dt == mybir.dt.bfloat16:
        import ml_dtypes
        return ml_dtypes.bfloat16
    return np.float32


def _perm(c):
    """Rotation putting core c's own queries at m-tiles OWN_M, OWN_M+1."""
    return np.roll(np.arange(B), OWN_M * 128 - c * QSLAB)


def prepare(q, p, mm_dt=None):
    """Host-side shard prep. Returns (in_maps, t32, perms)."""
    if mm_dt is None:
        mm_dt = MM_DT
    npdt = _np_dtype(mm_dt)
    q = np.ascontiguousarray(np.asarray(q, dtype=np.float32))
    p = np.ascontiguousarray(np.asarray(p, dtype=np.float32))

    # target scores t_i = q_i . p_{8i} (fp32; matches the reference's fp32
    # value to ~1e-7 — only a compare threshold + host-tail term)
    t32 = np.einsum("ij,ij->i", q, p[::NP], dtype=np.float64).astype(np.float32)

    qT = np.ascontiguousarray(q.T)  # [D, B] fp32
    r = np.arange(128)
    # self columns: unit 2*OWN_M has query pi=OWN_M*128+r vs local col 8r
    # (half 0); unit 2*OWN_M+3 has pi=(OWN_M+1)*128+r vs col 1024+8r
    # (i.e. col 8r of half 1). Same mask for both, same for every core.
    msk = np.ones((128, 1024), dtype=np.float32)
    msk[r, 8 * r] = 0.0

    in_maps = []
    perms = []
    for c in range(NCORES):
        perm = _perm(c)
        perms.append(perm)
        qTc = np.ascontiguousarray(qT[:, perm]).astype(npdt)
        pTc = np.ascontiguousarray(p[c * PSLAB:(c + 1) * PSLAB].T).astype(npdt)
        tvc = np.ascontiguousarray(t32[perm].reshape(MT, 128).T)
        in_maps.append({"qT": qTc, "pT": pTc, "tvec": tvc, "msk": msk})
    return in_maps, t32, perms


def finalize(results, t32, perms):
    """Combine per-core partials into the scalar loss (fp64 host tail)."""
    se_tot = np.zeros(B, dtype=np.float64)
    cnt_tot = np.zeros(B, dtype=np.float64)
    for c in range(NCORES):
        perm = perms[c]
        # column u = nh*MT + m; query pi = m*128 + r
        se = results[c]["se_out"].astype(np.float64)
        cnt = results[c]["cnt_out"].astype(np.float64)
        se_q = (se[:, :MT] + se[:, MT:]).T.ravel()
        cnt_q = (cnt[:, :MT] + cnt[:, MT:]).T.ravel()
        se_tot[perm] += se_q
        cnt_tot[perm] += cnt_q
    lse = C_SHIFT + np.log(se_tot)
    raw = lse - t32.astype(np.float64)
    w = 1.0 + ALPHA * np.exp(-((cnt_tot - OPTIMAL_RANK) ** 2)
                             / (2.0 * SIGMA ** 2))
    return np.float32(np.mean(raw * w))


def _get_nc(mm_dt=None):
    if mm_dt is None:
        mm_dt = MM_DT
    if mm_dt not in _STATE:
        _STATE[mm_dt] = _build_nc(mm_dt)
    return _STATE[mm_dt]


def kernel(q_reps, p_reps, n_passages):
    assert int(np.asarray(n_passages)) == NP
    nc = _get_nc()
    in_maps, t32, perms = prepare(q_reps, p_reps)
    res = run_bass_kernel_spmd(nc, in_maps, core_ids=list(range(NCORES)))
    return finalize(res.results, t32, perms)


def run_profiled(q_reps, p_reps, n_passages, mm_dt=None, trace=True):
    """Same as kernel() but returns (loss, BassKernelResults) with NTFF
    profile (requires the antenv.axon_hooks shim; see _install_ntff_shim)."""
    nc = _get_nc(mm_dt)
    in_maps, t32, perms = prepare(q_reps, p_reps, mm_dt)
    res = run_bass_kernel_spmd(nc, in_maps, core_ids=list(range(NCORES)),
                               trace=trace)
    loss = finalize(res.results, t32, perms)
    return loss, res


def _install_ntff_shim():
    """Provide antenv.axon_hooks (absent in this image) so trace=True works."""
    import types
    import antenv
    if "antenv.axon_hooks" in sys.modules:
        return
    mod = types.ModuleType("antenv.axon_hooks")
    mod._hook = None
    mod.set_axon_ntff_profile_hook = lambda h: setattr(mod, "_hook", h)
    mod.get_axon_ntff_profile_hook = lambda: mod._hook
    sys.modules["antenv.axon_hooks"] = mod
    antenv.axon_hooks = mod
    try:
        from trn_agent_boot.trn_boot import _ntff_profile_via_ctypes
        hook = _ntff_profile_via_ctypes("/opt/axon/libaxon_pjrt.so")
        if hook is not None:
            mod._hook = hook
    except Exception:
        pass
